# revision 1
# baseline (speedup 1.0000x reference)
"""Trainium2 Bass kernel for segmented linear (performer-style) attention.

Problem: nn_Attention_43550968382196 (sparse_attention).
  N=32768 tokens in 64 contiguous equal segments of 512, d_qk=128, d_v=256,
  m=256 random features.  Per segment:
     phi_q = (exp(Uq - hq - rowmax(Uq)) + eps) / sqrt(m)
     phi_k = (exp(Uk - hk - segmax(Uk)) + eps) / sqrt(m)
     out   = (phi_q @ (phi_k^T V)) / (phi_q . sum(phi_k) + 1e-8)

Device math (all equivalent to the reference up to ~1e-6):
  * 1/sqrt(m) cancels in the ratio -> unscaled phi, eps_norm' = 1e-8*m.
  * exp runs BEFORE the max; rowmax(exp(U)) == exp(rowmax U) by
    monotonicity, so the stabilizer is a multiplicative per-row scale.
  * The K side is left unnormalized by the segment max (it cancels in the
    ratio): Kp~ = exp(Uk)*exp(-hk).  The phi-eps term then needs
    eps*segmax, which is folded in as a rank-1 PE-accumulated correction
    KV += (eps*segmax) * ones ⊗ colsum([V|1]), and the normalizer eps
    becomes (1e-8*m)*segmax, broadcast via a tiny ones-matmul.  This keeps
    the cross-partition segment-max reduction off the critical path.
  * fp32r (11-bit-mantissa fp32) operands for all big matmuls; V/omega/Q^T/
    K^T are pre-rounded on the host, Qp/Kp/KV round on the producing engine.

Sharding: 64 segments split 8-per-core across 8 NeuronCores (data parallel,
no collectives); each core runs this program on its 4096-token shard.
"""

import math
import os
import sys

for _p in ("/opt/trn_rl_repo",):
    if _p not in sys.path and os.path.isdir(_p):
        sys.path.insert(0, _p)

import numpy as np

import concourse.bass as bass
import concourse.bacc as bacc
import concourse.tile as tile
from concourse import mybir
from concourse.bass_utils import run_bass_kernel_spmd

F32 = mybir.dt.float32
F32R = mybir.dt.float32r
AF = mybir.ActivationFunctionType
ALU = mybir.AluOpType
AX = mybir.AxisListType

N_CORES = 8
N = 32768
D = 128          # qk dim
M = 256          # features
DV = 256         # v dim
DVA = 258        # device V columns: [V | 1 | 0] (fp32r needs even N)
P = 128          # partitions / tokens per chunk
NSEG = int(os.environ.get('KERNEL_NSEG', 8))  # segments per core
CH = 4           # chunks per segment
MC = 2           # m chunks (256 / 128)
TOK = NSEG * 512

EPS_PHI = 1e-4
EPS_NORM2 = 1e-8 * M
H_SCALE = 1.0 / (2.0 * math.sqrt(D))
SQ2 = H_SCALE ** 0.5          # Square(x*SQ2) = x^2/(2 sqrt d)


def build_nc():
    nc = bacc.Bacc("TRN2", target_bir_lowering=False, debug=False)

    HQd = nc.declare_dram_parameter("HQK", [P, NSEG * CH * 2], F32,
                                    isOutput=False)
    QTd = nc.declare_dram_parameter("QT", [D, TOK], F32R, isOutput=False)
    KTd = nc.declare_dram_parameter("KT", [D, TOK], F32R, isOutput=False)
    Vd = nc.declare_dram_parameter("V", [TOK, DVA], F32R, isOutput=False)
    Wd = nc.declare_dram_parameter("omega", [D, M], F32R, isOutput=False)
    Id = nc.declare_dram_parameter("ident", [P, P], F32, isOutput=False)
    Ir = nc.declare_dram_parameter("identr", [P, P], F32R, isOutput=False)
    Ord = nc.declare_dram_parameter("onesr", [1, P], F32R, isOutput=False)
    Ocd = nc.declare_dram_parameter("onesc", [P, 1], F32R, isOutput=False)
    Od = nc.declare_dram_parameter("out", [TOK, DV], F32, isOutput=True)

    Vv = Vd[:, :].rearrange("(s c p) d -> s p c d", s=NSEG, c=CH, p=P)
    Ov = Od[:, :].rearrange("(s c p) d -> s p c d", s=NSEG, c=CH, p=P)

    with tile.TileContext(nc) as tc:
        with (
            tc.tile_pool(name="const", bufs=1) as const,
            tc.tile_pool(name="sb", bufs=2) as sb,
            tc.tile_pool(name="sm", bufs=3) as sm,
            tc.tile_pool(name="ps", bufs=1, space="PSUM") as ps,
        ):
            omega_t = const.tile([D, M], F32R, name="omega_t")
            nc.sync.dma_start(omega_t[:, :], Wd[:, :])
            ident_t = const.tile([P, P], F32, name="ident_t")
            nc.sync.dma_start(ident_t[:, :], Id[:, :])
            ident_r = const.tile([P, P], F32R, name="ident_r")
            nc.sync.dma_start(ident_r[:, :], Ir[:, :])
            ones_row = const.tile([1, P], F32, name="ones_row")
            nc.vector.memset(ones_row[:, :], 1.0)
            onesr_t = const.tile([1, P], F32R, name="onesr_t")
            nc.sync.dma_start(onesr_t[:, :], Ord[:, :])
            onesc_t = const.tile([P, 1], F32R, name="onesc_t")
            nc.sync.dma_start(onesc_t[:, :], Ocd[:, :])
            # per-segment slices so segment 0 compute starts right away
            qT_all = const.tile([D, TOK], F32R, name="qT_all")
            kT_all = const.tile([D, TOK], F32R, name="kT_all")
            for s in range(NSEG):
                sl = bass.ts(s, 512)
                nc.sync.dma_start(qT_all[:, sl], QTd[:, sl])
                nc.sync.dma_start(kT_all[:, sl], KTd[:, sl])
            hq_all = const.tile([P, NSEG, CH, 2], F32, name="hq_all")
            nc.sync.dma_start(
                hq_all[:, :, :, :],
                HQd[:, :].rearrange("p (s c t) -> p s c t", s=NSEG, c=CH))


            for s in range(NSEG):
                vt = sb.tile([P, CH, DVA], F32R, name=f"vt{s}", tag="vt",
                             bufs=4)
                nc.sync.dma_start(vt[:, :, :], Vv[s])
                hqk = hq_all[:, s]

                # ---- U matmuls (lhsT slices of preloaded Q^T/K^T) -------
                uq0 = ps.tile([P, 2, M], F32, name=f"uq0_{s}", tag="U", bufs=3)
                uq1 = ps.tile([P, 2, M], F32, name=f"uq1_{s}", tag="U", bufs=3)
                uk0 = ps.tile([P, 2, M], F32, name=f"uk0_{s}", tag="U", bufs=3)
                uk1 = ps.tile([P, 2, M], F32, name=f"uk1_{s}", tag="U", bufs=3)
                uqh = (uq0, uq1)
                ukh = (uk0, uk1)
                for c in range(CH):
                    nc.tensor.matmul(uqh[c // 2][:, c % 2, :],
                                     qT_all[:, bass.ts(s * CH + c, P)],
                                     omega_t[:, :])
                    nc.tensor.matmul(ukh[c // 2][:, c % 2, :],
                                     kT_all[:, bass.ts(s * CH + c, P)],
                                     omega_t[:, :])

                # ---- exp: eq0 = exp(Uq) raw; ek1 = exp(Uk - hk) ---------
                eq0 = sb.tile([P, CH, M], F32, name=f"eq0_{s}", tag="eq0", bufs=4)
                for hf in range(2):
                    nc.scalar.activation(eq0[:, 2 * hf:2 * hf + 2, :],
                                         uqh[hf][:, :, :], AF.Exp)
                ek1 = sb.tile([P, CH, M], F32R, name=f"ek1_{s}", tag="ek1", bufs=4)
                for c in range(CH):
                    nc.scalar.activation(ek1[:, c, :], ukh[c // 2][:, c % 2, :],
                                         AF.Exp, bias=hqk[:, c, 1:2])

                # ---- maxes from raw U (PSUM) ----------------------------
                xmq = sm.tile([P, CH], F32, name=f"xmq{s}", tag="xmq")
                nc.vector.tensor_reduce(xmq[:, 0:2], uq0[:, :, :],
                                        axis=AX.X, op=ALU.max)
                nc.vector.tensor_reduce(xmq[:, 2:4], uq1[:, :, :],
                                        axis=AX.X, op=ALU.max)
                xmk2 = sm.tile([P, 2], F32, name=f"xmk2_{s}", tag="xmk2")
                nc.vector.tensor_reduce(xmk2[:, 0:1], uk0[:, :, :],
                                        axis=AX.XY, op=ALU.max)
                nc.vector.tensor_reduce(xmk2[:, 1:2], uk1[:, :, :],
                                        axis=AX.XY, op=ALU.max)
                xmk = sm.tile([P, 1], F32, name=f"xmk{s}", tag="xmk")
                nc.vector.tensor_tensor(xmk[:, :], xmk2[:, 0:1],
                                        xmk2[:, 1:2], op=ALU.max)
                # segment max -> scalar (PE transpose + reduce); feeds only
                # the eps corrections, off the critical path
                mkT = ps.tile([1, 512], F32, name=f"mkT{s}", tag="S", bufs=1)
                nc.tensor.transpose(mkT[0:1, 0:P], xmk[:, 0:1], ident_t[:, :])
                mkrow = sm.tile([1, P], F32, name=f"mkrow{s}", tag="mkrow")
                nc.vector.tensor_copy(mkrow[:, :], mkT[0:1, 0:P])
                msr = sm.tile([1, 1], F32, name=f"msr{s}", tag="msr")
                nc.vector.tensor_reduce(msr[:, :], mkrow[:, :], axis=AX.X,
                                        op=ALU.max)
                mks = sm.tile([1, 1], F32, name=f"mks{s}", tag="mks")
                nc.scalar.activation(mks[:, :], msr[:, :], AF.Exp)

                # Vsum = colsum([V|1|0]) via ones-column matmul (PE)
                vsum = ps.tile([1, 512], F32, name=f"vsum{s}", tag="S",
                               bufs=1)
                for c in range(CH):
                    nc.tensor.matmul(vsum[0:1, 0:DVA], vt[:, c, DV:DV + 1],
                                     vt[:, c, :], start=(c == 0),
                                     stop=(c == CH - 1))
                # cvs = (eps_phi * segmax) * Vsum   [1, DVA] fp32r
                ceps = sm.tile([1, 1], F32, name=f"ceps{s}", tag="ceps")
                nc.vector.tensor_scalar_mul(ceps[:, :], mks[:, :], EPS_PHI)
                cvs = sm.tile([1, DVA], F32R, name=f"cvs{s}", tag="cvs")
                nc.vector.tensor_scalar_mul(cvs[:, :], vsum[0:1, 0:DVA],
                                            ceps[0:1, 0:1])
                # eps_norm * segmax broadcast to all partitions (PE)
                cen = sm.tile([1, 1], F32, name=f"cen{s}", tag="cen")
                nc.vector.tensor_scalar_mul(cen[:, :], mks[:, :], EPS_NORM2)
                enb = ps.tile([P, 512], F32, name=f"enb{s}", tag="S", bufs=1)
                nc.tensor.matmul(enb[:, 0:1], ones_row[:, :], cen[:, :])
                enb_sb = sm.tile([P, 1], F32, name=f"enbsb{s}", tag="enbsb")
                nc.vector.tensor_copy(enb_sb[:, :], enb[:, 0:1])

                # ---- Qp = eq0 * exp(-hq - mq) + eps ---------------------
                sqa = sm.tile([P, CH], F32, name=f"sqa{s}", tag="sqa")
                nc.vector.tensor_tensor(sqa[:, :], hqk[:, :, 0], xmq[:, :],
                                        op=ALU.subtract)
                sqv = sm.tile([P, CH], F32, name=f"sqv{s}", tag="sqv")
                nc.scalar.activation(sqv[:, :], sqa[:, :], AF.Exp)
                qp = sb.tile([P, CH, M], F32R, name=f"qp{s}", tag="qp", bufs=4)
                for c in range(CH):
                    nc.vector.tensor_scalar(qp[:, c, :], eq0[:, c, :],
                                            sqv[:, c:c + 1], EPS_PHI,
                                            op0=ALU.mult, op1=ALU.add)

                # ---- KV = Kp~^T @ [V|1|0]  (+ rank-1 eps correction) ----
                kv_sb = sb.tile([P, MC, DVA], F32R, name=f"kvsb{s}",
                                tag="kvsb", bufs=4)
                for mc in range(MC):
                    kvp = ps.tile([P, 512], F32, name=f"kv{s}_{mc}", tag="W",
                                  bufs=4)
                    for c in range(CH):
                        nc.tensor.matmul(kvp[:, 0:DVA],
                                         ek1[:, c, bass.ts(mc, P)],
                                         vt[:, c, :],
                                         start=(c == 0), stop=False)
                    nc.tensor.matmul(kvp[:, 0:DVA], onesr_t[0:1, :],
                                     cvs[0:1, :], start=False, stop=True)
                    if (mc + s) % 2 == 0:
                        nc.scalar.copy(kv_sb[:, mc, :], kvp[:, 0:DVA])
                    else:
                        nc.vector.tensor_copy(kv_sb[:, mc, :], kvp[:, 0:DVA])

                # ---- Qp^T (PE transpose) --------------------------------
                qpT_sb = sb.tile([P, MC, 512], F32R, name=f"qpTsb{s}",
                                 tag="qpTsb", bufs=4)
                for mc in range(MC):
                    qpTp = ps.tile([P, 512], F32R, name=f"qpT{s}_{mc}",
                                   tag="W", bufs=4)
                    for c in range(CH):
                        nc.tensor.transpose(qpTp[:, bass.ts(c, P)],
                                            qp[:, c, bass.ts(mc, P)],
                                            ident_r[:, :])
                    nc.scalar.copy(qpT_sb[:, mc, :], qpTp[:, :])

                # ---- num = Qp @ [KV | Ksum], per chunk ------------------
                ot = sb.tile([P, CH, DV], F32, name=f"ot{s}", tag="ot",
                             bufs=4)
                for c in range(CH):
                    nm = ps.tile([P, 512], F32, name=f"nm{s}_{c}",
                                 tag="W", bufs=4)
                    for mc in range(MC):
                        nc.tensor.matmul(nm[:, 0:DVA],
                                         qpT_sb[:, mc, bass.ts(c, P)],
                                         kv_sb[:, mc, :],
                                         start=(mc == 0),
                                         stop=(mc == MC - 1))
                    den = sm.tile([P, 1], F32, name=f"den{s}_{c}", tag="den")
                    nc.vector.tensor_scalar_add(den[:, :],
                                                nm[:, DV:DV + 1],
                                                enb_sb[:, 0:1])
                    rr = sm.tile([P, 1], F32, name=f"rr{s}_{c}", tag="rr")
                    nc.vector.reciprocal(rr[:, :], den[:, :])
                    if (c + s) % 2 == 0:
                        rrb = rr[:, :].broadcast_to([P, DV])
                        nc.vector.tensor_tensor(
                            ot[:, c, :], nm[:, 0:DV], rrb, op=ALU.mult)
                    else:
                        nc.scalar.activation(ot[:, c, :], nm[:, 0:DV],
                                             AF.Copy, scale=rr[:, 0:1])

                nc.sync.dma_start(Ov[s], ot[:, :, :])

    nc.compile()
    return nc


_NC_CACHE = {}


def _get_nc():
    if "nc" not in _NC_CACHE:
        _NC_CACHE["nc"] = build_nc()
    return _NC_CACHE["nc"]


def _round_f32r(x):
    xi = np.ascontiguousarray(x, np.float32).view(np.uint32)
    return ((xi + np.uint32(1 << 11)) & np.uint32(0xFFFFF000)).view(np.float32)


def make_in_maps(Q, K, V, omega):
    Q = np.ascontiguousarray(np.asarray(Q, dtype=np.float32))
    K = np.ascontiguousarray(np.asarray(K, dtype=np.float32))
    QT = _round_f32r(Q.T)
    KT = _round_f32r(K.T)
    hscale = np.float32(1.0 / (2.0 * math.sqrt(D)))
    hq = -(Q * Q).sum(axis=1) * hscale
    hk = -(K * K).sum(axis=1) * hscale
    # device layout [P, (s c t)] with token = (s*CH + c)*P + p per core
    hqk2 = np.stack([hq, hk], axis=1)          # [N, 2]
    V = np.asarray(V, dtype=np.float32)
    Vaug = np.zeros((V.shape[0], DVA), np.float32)
    Vaug[:, :DV] = _round_f32r(V)
    Vaug[:, DV] = 1.0
    omega = np.asarray(omega, dtype=np.float32)
    omega_s = _round_f32r(omega * np.float32(D ** -0.25))
    ident = np.eye(P, dtype=np.float32)
    ones_r = np.ones((1, P), np.float32)
    ones_c = np.ones((P, 1), np.float32)
    in_maps = []
    for c in range(N_CORES):
        sl = slice(c * TOK, (c + 1) * TOK)
        in_maps.append({
            "V": Vaug[sl],
            "HQK": np.ascontiguousarray(
                hqk2[sl].reshape(NSEG, CH, P, 2)
                .transpose(2, 0, 1, 3).reshape(P, NSEG * CH * 2)),
            "QT": np.ascontiguousarray(QT[:, sl]),
            "KT": np.ascontiguousarray(KT[:, sl]),
            "omega": omega_s, "ident": ident, "identr": ident,
            "onesr": ones_r, "onesc": ones_c,
        })
    return in_maps


def kernel(Q, K, V, omega, num_batch, batch_seg):
    nc = _get_nc()
    in_maps = make_in_maps(Q, K, V, omega)
    res = run_bass_kernel_spmd(nc, in_maps, core_ids=list(range(N_CORES)))
    return np.concatenate([res.results[c]["out"] for c in range(N_CORES)],
                          axis=0)



# revision 8
# speedup vs baseline: 1.2026x; 1.2026x over previous
"""Trainium2 Bass kernel for segmented linear (performer-style) attention.

Problem: nn_Attention_43550968382196 (sparse_attention).
  N=32768 tokens in 64 contiguous equal segments of 512, d_qk=128, d_v=256,
  m=256 random features.  Per segment:
     phi_q = (exp(Uq - hq - rowmax(Uq)) + eps) / sqrt(m)
     phi_k = (exp(Uk - hk - segmax(Uk)) + eps) / sqrt(m)
     out   = (phi_q @ (phi_k^T V)) / (phi_q . sum(phi_k) + 1e-8)

Device math (equivalent to the reference up to rounding):
  * 1/sqrt(m) cancels in the ratio -> unscaled phi, eps_norm' = 1e-8*m.
  * Both exps run RAW (no bias): exp is monotone, so
    rowmax(exp U) == exp(rowmax U) and the stabilizers become
    multiplicative post-factors.
  * Q side: qp = exp(Uq) * s + eps with s = exp(-hq)/rowmax(exp Uq)
    (one fused tensor_scalar per chunk).
  * K side: exp(-hk) is folded into V on the HOST (V' = exp(-hk)*[V|1|0]),
    so KV = exp(Uk)^T @ V' = (exp(Uk-hk))^T [V|1|0] directly.  The
    segment-max normalizer cancels in the ratio; the phi-eps term needs
    eps*exp(segmax), added as a rank-1 PE-accumulated correction
    KV += (eps*g) * ones (x) Vsum with Vsum precomputed on the host and
    g = allreduce-max(exp Uk) from a GpSimd partition all-reduce.  The
    normalizer eps becomes (1e-8*m)*g, added per row.
  * All big matmuls run in bf16 (host-prerounded inputs; device exps and
    copies produce bf16 operands); PSUM accumulation stays fp32.

Sharding: 64 segments split 8-per-core across 8 NeuronCores (data parallel,
no collectives).  Per core the 8 segments run through a 2-deep software
pipeline: iteration s queues U-matmuls(s), KV+transpose(s-1), output(s-2)
back-to-back on the PE so it never waits on the exp/copy chains.
"""

import math
import os
import sys

for _p in ("/opt/trn_rl_repo",):
    if _p not in sys.path and os.path.isdir(_p):
        sys.path.insert(0, _p)

import numpy as np
import ml_dtypes

import concourse.bass as bass
import concourse.bacc as bacc
import concourse.tile as tile
from concourse import mybir
from concourse import bass_isa
from concourse.bass_utils import run_bass_kernel_spmd

F32 = mybir.dt.float32
BF16 = mybir.dt.bfloat16
AF = mybir.ActivationFunctionType
ALU = mybir.AluOpType
AX = mybir.AxisListType
RED = bass_isa.ReduceOp

N_CORES = 8
N = 32768
D = 128          # qk dim
M = 256          # features
DV = 256         # v dim
DVA = 258        # device V columns: [V | 1 | 0]
P = 128          # partitions / tokens per chunk
NSEG = 8         # segments per core
CH = 4           # chunks per segment
MC = 2           # m chunks (256 / 128)
TOK = NSEG * 512

EPS_PHI = 1e-4
EPS_NORM2 = 1e-8 * M
H_SCALE = 1.0 / (2.0 * math.sqrt(D))


def build_nc():
    nc = bacc.Bacc("TRN2", target_bir_lowering=False, debug=False)

    EHQd = nc.declare_dram_parameter("EHQ", [P, NSEG * CH], F32,
                                     isOutput=False)
    QTd = nc.declare_dram_parameter("QT", [D, TOK], BF16, isOutput=False)
    KTd = nc.declare_dram_parameter("KT", [D, TOK], BF16, isOutput=False)
    Vd = nc.declare_dram_parameter("V", [TOK, DVA], BF16, isOutput=False)
    Wd = nc.declare_dram_parameter("omega", [D, M], BF16, isOutput=False)
    Ird = nc.declare_dram_parameter("identr", [P, P], BF16, isOutput=False)
    Ord = nc.declare_dram_parameter("onesr", [1, P], BF16, isOutput=False)
    VSd = nc.declare_dram_parameter("vsum", [1, NSEG * DVA], F32,
                                    isOutput=False)
    Od = nc.declare_dram_parameter("out", [TOK, DV], F32, isOutput=True)

    Vv = Vd[:, :].rearrange("(s c p) d -> s p c d", s=NSEG, c=CH, p=P)
    Ov = Od[:, :].rearrange("(s c p) d -> s p c d", s=NSEG, c=CH, p=P)

    with tile.TileContext(nc) as tc:
        with (
            tc.tile_pool(name="const", bufs=1) as const,
            tc.tile_pool(name="sb", bufs=2) as sb,
            tc.tile_pool(name="sm", bufs=4) as sm,
            tc.tile_pool(name="ps", bufs=1, space="PSUM") as ps,
        ):
            omega_t = const.tile([D, M], BF16, name="omega_t")
            nc.sync.dma_start(omega_t[:, :], Wd[:, :])
            ident_r = const.tile([P, P], BF16, name="ident_r")
            nc.sync.dma_start(ident_r[:, :], Ird[:, :])
            onesr_t = const.tile([1, P], BF16, name="onesr_t")
            nc.sync.dma_start(onesr_t[:, :], Ord[:, :])
            ehq_all = const.tile([P, NSEG, CH], F32, name="ehq_all")
            nc.sync.dma_start(
                ehq_all[:, :, :],
                EHQd[:, :].rearrange("p (s c) -> p s c", s=NSEG))
            vsum_all = const.tile([1, NSEG, DVA], F32, name="vsum_all")
            nc.sync.dma_start(
                vsum_all[:, :, :],
                VSd[:, :].rearrange("p (s d) -> p s d", s=NSEG))
            # per-segment slices so segment 0 compute starts right away
            qT_all = const.tile([D, TOK], BF16, name="qT_all")
            kT_all = const.tile([D, TOK], BF16, name="kT_all")
            for s in range(NSEG):
                sl = bass.ts(s, 512)
                nc.sync.dma_start(kT_all[:, sl], KTd[:, sl])
                nc.sync.dma_start(qT_all[:, sl], QTd[:, sl])

            stA = {}
            stB = {}

            def head(s):
                """U matmuls + exps + maxes + eps factors for segment s."""
                vt = sb.tile([P, CH, DVA], BF16, name=f"vt{s}", tag="vt",
                             bufs=3)
                nc.sync.dma_start(vt[:, :, :], Vv[s])

                uk = ps.tile([P, CH, M], F32, name=f"uk{s}", tag="uk", bufs=1)
                uq = ps.tile([P, CH, M], F32, name=f"uq{s}", tag="uq", bufs=1)
                for c in range(CH):
                    nc.tensor.matmul(uk[:, c, :],
                                     kT_all[:, bass.ts(s * CH + c, P)],
                                     omega_t[:, :])
                for c in range(CH):
                    nc.tensor.matmul(uq[:, c, :],
                                     qT_all[:, bass.ts(s * CH + c, P)],
                                     omega_t[:, :])

                ek = sb.tile([P, CH, M], BF16, name=f"ek{s}", tag="ek",
                             bufs=3)
                nc.scalar.activation(ek[:, :, :], uk[:, :, :], AF.Exp)
                eq = sb.tile([P, CH, M], BF16, name=f"eq{s}", tag="eq",
                             bufs=3)
                nc.scalar.activation(eq[:, :, :], uq[:, :, :], AF.Exp)

                # K segment max -> g = exp(segmax) on every partition
                kmx = sm.tile([P, 1], F32, name=f"kmx{s}", tag="kmx")
                nc.vector.tensor_reduce(kmx[:, :], ek[:, :, :], axis=AX.XY,
                                        op=ALU.max)
                gmax = sm.tile([P, 1], F32, name=f"gmax{s}", tag="gmax")
                nc.gpsimd.partition_all_reduce(gmax[:, 0:1], kmx[:, 0:1],
                                               channels=P, reduce_op=RED.max)
                cen = sm.tile([P, 1], F32, name=f"cen{s}", tag="cen")
                nc.vector.tensor_scalar_mul(cen[:, :], gmax[:, :], EPS_NORM2)
                cvs = sm.tile([1, DVA], BF16, name=f"cvs{s}", tag="cvs")
                nc.vector.tensor_scalar(cvs[:, :], vsum_all[0:1, s, :],
                                        gmax[0:1, 0:1], EPS_PHI,
                                        op0=ALU.mult, op1=ALU.mult)

                # Q per-row scale s = exp(-hq) / rowmax(exp Uq)
                qmx = sm.tile([P, CH], BF16, name=f"qmx{s}", tag="qmx")
                nc.vector.tensor_reduce(qmx[:, :], eq[:, :, :], axis=AX.X,
                                        op=ALU.max)
                rq = sm.tile([P, CH], F32, name=f"rq{s}", tag="rq")
                nc.vector.reciprocal(rq[:, :], qmx[:, :])
                sq = sm.tile([P, CH], F32, name=f"sq{s}", tag="sq")
                nc.vector.tensor_tensor(sq[:, :], ehq_all[:, s, :], rq[:, :],
                                        op=ALU.mult)
                qp = sb.tile([P, CH, M], BF16, name=f"qp{s}", tag="qp",
                             bufs=3)
                for c in range(CH):
                    nc.vector.tensor_scalar(qp[:, c, :], eq[:, c, :],
                                            sq[:, c:c + 1], EPS_PHI,
                                            op0=ALU.mult, op1=ALU.add)
                stA[s] = (vt, ek, qp, cvs, cen)

            def mid(s):
                """KV matmuls + Qp transposes for segment s."""
                vt, ek, qp, cvs, cen = stA.pop(s)
                kv_sb = sb.tile([P, MC, DVA], BF16, name=f"kvsb{s}",
                                tag="kvsb", bufs=3)
                qpT_sb = sb.tile([P, MC, 512], BF16, name=f"qpTsb{s}",
                                 tag="qpTsb", bufs=3)
                for mc in range(MC):
                    kvp = ps.tile([P, 512], F32, name=f"kv{s}_{mc}", tag="W",
                                  bufs=4)
                    for c in range(CH):
                        nc.tensor.matmul(kvp[:, 0:DVA],
                                         ek[:, c, bass.ts(mc, P)],
                                         vt[:, c, :],
                                         start=(c == 0), stop=False)
                    nc.tensor.matmul(kvp[:, 0:DVA], onesr_t[0:1, :],
                                     cvs[0:1, :], start=False, stop=True)
                    nc.vector.tensor_copy(kv_sb[:, mc, :], kvp[:, 0:DVA])
                    qpTp = ps.tile([P, 512], BF16, name=f"qpT{s}_{mc}",
                                   tag="W", bufs=4)
                    for c in range(CH):
                        nc.tensor.transpose(qpTp[:, bass.ts(c, P)],
                                            qp[:, c, bass.ts(mc, P)],
                                            ident_r[:, :])
                    if mc == 0:
                        nc.scalar.copy(qpT_sb[:, mc, :], qpTp[:, :])
                    else:
                        nc.vector.tensor_copy(qpT_sb[:, mc, :], qpTp[:, :])
                stB[s] = (kv_sb, qpT_sb, cen)

            def tail(s):
                """Numerator, normalization and output for segment s."""
                kv_sb, qpT_sb, cen = stB.pop(s)
                ot = sb.tile([P, CH, DV], F32, name=f"ot{s}", tag="ot",
                             bufs=3)
                for c in range(CH):
                    nm = ps.tile([P, 512], F32, name=f"nm{s}_{c}",
                                 tag="W", bufs=4)
                    for mc in range(MC):
                        nc.tensor.matmul(nm[:, 0:DVA],
                                         qpT_sb[:, mc, bass.ts(c, P)],
                                         kv_sb[:, mc, :],
                                         start=(mc == 0),
                                         stop=(mc == MC - 1))
                    den = sm.tile([P, 1], F32, name=f"den{s}_{c}", tag="den")
                    nc.vector.tensor_tensor(den[:, :], nm[:, DV:DV + 1],
                                            cen[:, 0:1], op=ALU.add)
                    rr = sm.tile([P, 1], F32, name=f"rr{s}_{c}", tag="rr")
                    nc.vector.reciprocal(rr[:, :], den[:, :])
                    nc.scalar.activation(ot[:, c, :], nm[:, 0:DV],
                                         AF.Copy, scale=rr[:, 0:1])
                nc.sync.dma_start(Ov[s], ot[:, :, :])

            # 2-deep software pipeline (see module docstring)
            for s in range(NSEG):
                head(s)
                if s >= 1:
                    mid(s - 1)
                if s >= 2:
                    tail(s - 2)
            mid(NSEG - 1)
            tail(NSEG - 2)
            tail(NSEG - 1)

    nc.compile()
    return nc


_NC_CACHE = {}


def _get_nc():
    if "nc" not in _NC_CACHE:
        _NC_CACHE["nc"] = build_nc()
    return _NC_CACHE["nc"]


def _bf16(x):
    return np.ascontiguousarray(x.astype(ml_dtypes.bfloat16))


def make_in_maps(Q, K, V, omega):
    Q = np.ascontiguousarray(np.asarray(Q, dtype=np.float32))
    K = np.ascontiguousarray(np.asarray(K, dtype=np.float32))
    QT = _bf16(Q.T)
    KT = _bf16(K.T)
    hscale = np.float32(H_SCALE)
    ehq = np.exp(-(Q * Q).sum(axis=1) * hscale)          # exp(-hq)  [N]
    ehk = np.exp(-(K * K).sum(axis=1) * hscale)          # exp(-hk)  [N]
    V = np.asarray(V, dtype=np.float32)
    Vaug = np.zeros((V.shape[0], DVA), np.float32)
    Vaug[:, :DV] = V
    Vaug[:, DV] = 1.0
    # per-segment column sums of [V | 1 | 0] (fp32, host-side)
    vsum = Vaug.reshape(N // 512, 512, DVA).sum(axis=1)  # [64, DVA]
    Vp16 = _bf16(Vaug * ehk[:, None])                    # exp(-hk)-folded V
    omega = np.asarray(omega, dtype=np.float32)
    omega16 = _bf16(omega * np.float32(D ** -0.25))
    ident16 = np.eye(P, dtype=ml_dtypes.bfloat16)
    ones_r16 = np.ones((1, P), ml_dtypes.bfloat16)
    in_maps = []
    for c in range(N_CORES):
        sl = slice(c * TOK, (c + 1) * TOK)
        in_maps.append({
            "V": Vp16[sl],
            "EHQ": np.ascontiguousarray(
                ehq[sl].reshape(NSEG, CH, P)
                .transpose(2, 0, 1).reshape(P, NSEG * CH)),
            "QT": np.ascontiguousarray(QT[:, sl]),
            "KT": np.ascontiguousarray(KT[:, sl]),
            "omega": omega16, "identr": ident16, "onesr": ones_r16,
            "vsum": np.ascontiguousarray(
                vsum[c * NSEG:(c + 1) * NSEG].reshape(1, NSEG * DVA)),
        })
    return in_maps


def kernel(Q, K, V, omega, num_batch, batch_seg):
    nc = _get_nc()
    in_maps = make_in_maps(Q, K, V, omega)
    res = run_bass_kernel_spmd(nc, in_maps, core_ids=list(range(N_CORES)))
    return np.concatenate([res.results[c]["out"] for c in range(N_CORES)],
                          axis=0)


# revision 13
# speedup vs baseline: 1.2777x; 1.0624x over previous
"""Trainium2 Bass kernel for segmented linear (performer-style) attention.

Problem: nn_Attention_43550968382196 (sparse_attention).
  N=32768 tokens in 64 contiguous equal segments of 512, d_qk=128, d_v=256,
  m=256 random features.  Per segment:
     phi_q = (exp(Uq - hq - rowmax(Uq)) + eps) / sqrt(m)
     phi_k = (exp(Uk - hk - segmax(Uk)) + eps) / sqrt(m)
     out   = (phi_q @ (phi_k^T V)) / (phi_q . sum(phi_k) + 1e-8)

Device math (equivalent to the reference up to rounding):
  * 1/sqrt(m) cancels in the ratio -> unscaled phi, eps_norm' = 1e-8*m.
  * Both exps run RAW (no bias): exp is monotone, so
    rowmax(exp U) == exp(rowmax U) and the stabilizers become
    multiplicative post-factors.
  * Q side: qp = exp(Uq) * s + eps with s = exp(-hq)/rowmax(exp Uq)
    (one fused tensor_scalar per chunk).
  * K side: exp(-hk) is folded into V on the HOST (V' = exp(-hk)*[V|1|0]),
    so KV = exp(Uk)^T @ V' = (exp(Uk-hk))^T [V|1|0] directly.  The
    segment-max normalizer cancels in the ratio; the phi-eps term needs
    eps*exp(segmax), added as a rank-1 PE-accumulated correction
    KV += (eps*g) * ones (x) Vsum with Vsum precomputed on the host and
    g = allreduce-max(exp Uk) from a GpSimd partition all-reduce.  The
    normalizer eps becomes (1e-8*m)*g, added per row.
  * All big matmuls run in bf16 (host-prerounded inputs; device exps and
    copies produce bf16 operands); PSUM accumulation stays fp32.

Sharding: 64 segments split 8-per-core across 8 NeuronCores (data parallel,
no collectives).  Per core the 8 segments run through a 2-deep software
pipeline: iteration s queues U-matmuls(s), KV+transpose(s-1), output(s-2)
back-to-back on the PE so it never waits on the exp/copy chains.
"""

import math
import os
import sys

for _p in ("/opt/trn_rl_repo",):
    if _p not in sys.path and os.path.isdir(_p):
        sys.path.insert(0, _p)

import numpy as np
import ml_dtypes

import concourse.bass as bass
import concourse.bacc as bacc
import concourse.tile as tile
from concourse import mybir
from concourse import bass_isa
from concourse.bass_utils import run_bass_kernel_spmd

F32 = mybir.dt.float32
BF16 = mybir.dt.bfloat16
AF = mybir.ActivationFunctionType
ALU = mybir.AluOpType
AX = mybir.AxisListType
RED = bass_isa.ReduceOp

N_CORES = 8
N = 32768
D = 128          # qk dim
M = 256          # features
DV = 256         # v dim
DVA = 258        # device V columns: [V | 1 | 0]
P = 128          # partitions / tokens per chunk
NSEG = 8         # segments per core
CH = 4           # chunks per segment
MC = 2           # m chunks (256 / 128)
TOK = NSEG * 512

EPS_PHI = 1e-4
EPS_NORM2 = 1e-8 * M
H_SCALE = 1.0 / (2.0 * math.sqrt(D))


def build_nc():
    nc = bacc.Bacc("TRN2", target_bir_lowering=False, debug=False)

    EHQd = nc.declare_dram_parameter("EHQ", [P, NSEG * CH], F32,
                                     isOutput=False)
    QTd = nc.declare_dram_parameter("QT", [D, TOK], BF16, isOutput=False)
    KTd = nc.declare_dram_parameter("KT", [D, TOK], BF16, isOutput=False)
    # V and out live in DEVICE layout in DRAM (host rearranges), so DMA
    # lines are >=2KB contiguous per partition.
    Vd = nc.declare_dram_parameter("V", [P, NSEG * CH * DVA], BF16,
                                   isOutput=False)
    Wd = nc.declare_dram_parameter("omega", [D, M], BF16, isOutput=False)
    Ird = nc.declare_dram_parameter("identr", [P, P], BF16, isOutput=False)
    Ord = nc.declare_dram_parameter("onesr", [1, P], BF16, isOutput=False)
    VSd = nc.declare_dram_parameter("vsum", [1, NSEG * DVA], F32,
                                    isOutput=False)
    Od = nc.declare_dram_parameter("out", [P, NSEG * CH * DV], BF16,
                                   isOutput=True)

    Vv = Vd[:, :].rearrange("p (s c d) -> s p c d", s=NSEG, c=CH)
    Ov = Od[:, :].rearrange("p (s c d) -> s p c d", s=NSEG, c=CH)

    with tile.TileContext(nc) as tc:
        with (
            tc.tile_pool(name="const", bufs=1) as const,
            tc.tile_pool(name="sb", bufs=2) as sb,
            tc.tile_pool(name="sm", bufs=4) as sm,
            tc.tile_pool(name="ps", bufs=1, space="PSUM") as ps,
        ):
            omega_t = const.tile([D, M], BF16, name="omega_t")
            nc.sync.dma_start(omega_t[:, :], Wd[:, :])
            ident_r = const.tile([P, P], BF16, name="ident_r")
            nc.sync.dma_start(ident_r[:, :], Ird[:, :])
            onesr_t = const.tile([1, P], BF16, name="onesr_t")
            nc.sync.dma_start(onesr_t[:, :], Ord[:, :])
            ehq_all = const.tile([P, NSEG, CH], F32, name="ehq_all")
            nc.sync.dma_start(
                ehq_all[:, :, :],
                EHQd[:, :].rearrange("p (s c) -> p s c", s=NSEG))
            vsum_all = const.tile([1, NSEG, DVA], F32, name="vsum_all")
            nc.sync.dma_start(
                vsum_all[:, :, :],
                VSd[:, :].rearrange("p (s d) -> p s d", s=NSEG))
            # halves (4KB contiguous per partition) so segment-0 compute
            # starts after ~1/4 of the Q/K traffic
            qT_all = const.tile([D, TOK], BF16, name="qT_all")
            kT_all = const.tile([D, TOK], BF16, name="kT_all")
            half = TOK // 2
            nc.sync.dma_start(kT_all[:, 0:half], KTd[:, 0:half])
            nc.sync.dma_start(qT_all[:, 0:half], QTd[:, 0:half])
            nc.sync.dma_start(kT_all[:, half:TOK], KTd[:, half:TOK])
            nc.sync.dma_start(qT_all[:, half:TOK], QTd[:, half:TOK])

            stA = {}
            stB = {}

            def head(s):
                """U matmuls + exps + maxes + eps factors for segment s."""
                vt = sb.tile([P, CH, DVA], BF16, name=f"vt{s}", tag="vt",
                             bufs=3)
                nc.sync.dma_start(vt[:, :, :], Vv[s])

                uk = ps.tile([P, CH, M], F32, name=f"uk{s}", tag="uk", bufs=1)
                uq = ps.tile([P, CH, M], F32, name=f"uq{s}", tag="uq", bufs=1)
                for c in range(CH):
                    nc.tensor.matmul(uk[:, c, :],
                                     kT_all[:, bass.ts(s * CH + c, P)],
                                     omega_t[:, :])
                for c in range(CH):
                    nc.tensor.matmul(uq[:, c, :],
                                     qT_all[:, bass.ts(s * CH + c, P)],
                                     omega_t[:, :])

                ek = sb.tile([P, CH, M], BF16, name=f"ek{s}", tag="ek",
                             bufs=3)
                nc.scalar.activation(ek[:, :, :], uk[:, :, :], AF.Exp)
                eq = sb.tile([P, CH, M], BF16, name=f"eq{s}", tag="eq",
                             bufs=3)
                nc.scalar.activation(eq[:, :, :], uq[:, :, :], AF.Exp)

                # K segment max -> g = exp(segmax) on every partition
                kmx = sm.tile([P, 1], F32, name=f"kmx{s}", tag="kmx")
                nc.vector.tensor_reduce(kmx[:, :], ek[:, :, :], axis=AX.XY,
                                        op=ALU.max)
                gmax = sm.tile([P, 1], F32, name=f"gmax{s}", tag="gmax")
                nc.gpsimd.partition_all_reduce(gmax[:, 0:1], kmx[:, 0:1],
                                               channels=P, reduce_op=RED.max)
                cen = sm.tile([P, 1], F32, name=f"cen{s}", tag="cen")
                nc.vector.tensor_scalar_mul(cen[:, :], gmax[:, :], EPS_NORM2)
                cvs = sm.tile([1, DVA], BF16, name=f"cvs{s}", tag="cvs")
                nc.vector.tensor_scalar(cvs[:, :], vsum_all[0:1, s, :],
                                        gmax[0:1, 0:1], EPS_PHI,
                                        op0=ALU.mult, op1=ALU.mult)

                # Q per-row scale s = exp(-hq) / rowmax(exp Uq)
                qmx = sm.tile([P, CH], BF16, name=f"qmx{s}", tag="qmx")
                nc.vector.tensor_reduce(qmx[:, :], eq[:, :, :], axis=AX.X,
                                        op=ALU.max)
                rq = sm.tile([P, CH], F32, name=f"rq{s}", tag="rq")
                nc.vector.reciprocal(rq[:, :], qmx[:, :])
                sq = sm.tile([P, CH], F32, name=f"sq{s}", tag="sq")
                nc.vector.tensor_tensor(sq[:, :], ehq_all[:, s, :], rq[:, :],
                                        op=ALU.mult)
                qp = sb.tile([P, CH, M], BF16, name=f"qp{s}", tag="qp",
                             bufs=3)
                for c in range(CH):
                    nc.vector.tensor_scalar(qp[:, c, :], eq[:, c, :],
                                            sq[:, c:c + 1], EPS_PHI,
                                            op0=ALU.mult, op1=ALU.add)
                stA[s] = (vt, ek, qp, cvs, cen)

            def mid(s):
                """KV matmuls + Qp transposes for segment s."""
                vt, ek, qp, cvs, cen = stA.pop(s)
                kv_sb = sb.tile([P, MC, DVA], BF16, name=f"kvsb{s}",
                                tag="kvsb", bufs=3)
                qpT_sb = sb.tile([P, MC, 512], BF16, name=f"qpTsb{s}",
                                 tag="qpTsb", bufs=3)
                for mc in range(MC):
                    kvp = ps.tile([P, 512], F32, name=f"kv{s}_{mc}", tag="W",
                                  bufs=4)
                    for c in range(CH):
                        nc.tensor.matmul(kvp[:, 0:DVA],
                                         ek[:, c, bass.ts(mc, P)],
                                         vt[:, c, :],
                                         start=(c == 0), stop=False)
                    nc.tensor.matmul(kvp[:, 0:DVA], onesr_t[0:1, :],
                                     cvs[0:1, :], start=False, stop=True)
                    nc.vector.tensor_copy(kv_sb[:, mc, :], kvp[:, 0:DVA])
                    qpTp = ps.tile([P, 512], BF16, name=f"qpT{s}_{mc}",
                                   tag="W", bufs=4)
                    for c in range(CH):
                        nc.tensor.transpose(qpTp[:, bass.ts(c, P)],
                                            qp[:, c, bass.ts(mc, P)],
                                            ident_r[:, :])
                    if mc == 0:
                        nc.scalar.copy(qpT_sb[:, mc, :], qpTp[:, :])
                    else:
                        nc.vector.tensor_copy(qpT_sb[:, mc, :], qpTp[:, :])
                stB[s] = (kv_sb, qpT_sb, cen)

            def tail(s):
                """Numerator, normalization and output for segment s."""
                kv_sb, qpT_sb, cen = stB.pop(s)
                ot = sb.tile([P, CH, DV], BF16, name=f"ot{s}", tag="ot",
                             bufs=3)
                for c in range(CH):
                    nm = ps.tile([P, 512], F32, name=f"nm{s}_{c}",
                                 tag="W", bufs=4)
                    for mc in range(MC):
                        nc.tensor.matmul(nm[:, 0:DVA],
                                         qpT_sb[:, mc, bass.ts(c, P)],
                                         kv_sb[:, mc, :],
                                         start=(mc == 0),
                                         stop=(mc == MC - 1))
                    den = sm.tile([P, 1], F32, name=f"den{s}_{c}", tag="den")
                    nc.vector.tensor_tensor(den[:, :], nm[:, DV:DV + 1],
                                            cen[:, 0:1], op=ALU.add)
                    rr = sm.tile([P, 1], F32, name=f"rr{s}_{c}", tag="rr")
                    nc.vector.reciprocal(rr[:, :], den[:, :])
                    nc.scalar.activation(ot[:, c, :], nm[:, 0:DV],
                                         AF.Copy, scale=rr[:, 0:1])
                nc.sync.dma_start(Ov[s], ot[:, :, :])

            # 2-deep software pipeline (see module docstring)
            for s in range(NSEG):
                head(s)
                if s >= 1:
                    mid(s - 1)
                if s >= 2:
                    tail(s - 2)
            mid(NSEG - 1)
            tail(NSEG - 2)
            tail(NSEG - 1)

    nc.compile()
    return nc


_NC_CACHE = {}


def _get_nc():
    if "nc" not in _NC_CACHE:
        _NC_CACHE["nc"] = build_nc()
    return _NC_CACHE["nc"]


def _bf16(x):
    return np.ascontiguousarray(x.astype(ml_dtypes.bfloat16))


def make_in_maps(Q, K, V, omega):
    Q = np.ascontiguousarray(np.asarray(Q, dtype=np.float32))
    K = np.ascontiguousarray(np.asarray(K, dtype=np.float32))
    QT = _bf16(Q.T)
    KT = _bf16(K.T)
    hscale = np.float32(H_SCALE)
    ehq = np.exp(-(Q * Q).sum(axis=1) * hscale)          # exp(-hq)  [N]
    ehk = np.exp(-(K * K).sum(axis=1) * hscale)          # exp(-hk)  [N]
    V = np.asarray(V, dtype=np.float32)
    Vaug = np.zeros((V.shape[0], DVA), np.float32)
    Vaug[:, :DV] = V
    Vaug[:, DV] = 1.0
    # per-segment column sums of [V | 1 | 0] (fp32, host-side)
    vsum = Vaug.reshape(N // 512, 512, DVA).sum(axis=1)  # [64, DVA]
    Vp16 = _bf16(Vaug * ehk[:, None])                    # exp(-hk)-folded V
    # device layout: [P, (s c d)] per core
    Vdev = Vp16.reshape(N_CORES, NSEG, CH, P, DVA).transpose(0, 3, 1, 2, 4)
    Vdev = np.ascontiguousarray(
        Vdev.reshape(N_CORES, P, NSEG * CH * DVA))
    omega = np.asarray(omega, dtype=np.float32)
    omega16 = _bf16(omega * np.float32(D ** -0.25))
    ident16 = np.eye(P, dtype=ml_dtypes.bfloat16)
    ones_r16 = np.ones((1, P), ml_dtypes.bfloat16)
    in_maps = []
    for c in range(N_CORES):
        sl = slice(c * TOK, (c + 1) * TOK)
        in_maps.append({
            "V": Vdev[c],
            "EHQ": np.ascontiguousarray(
                ehq[sl].reshape(NSEG, CH, P)
                .transpose(2, 0, 1).reshape(P, NSEG * CH)),
            "QT": np.ascontiguousarray(QT[:, sl]),
            "KT": np.ascontiguousarray(KT[:, sl]),
            "omega": omega16, "identr": ident16, "onesr": ones_r16,
            "vsum": np.ascontiguousarray(
                vsum[c * NSEG:(c + 1) * NSEG].reshape(1, NSEG * DVA)),
        })
    return in_maps


def unpack_out(res):
    # out arrives in device layout [P, (s c d)] bf16 per core
    outs = np.stack([np.asarray(res.results[c]["out"])
                     for c in range(N_CORES)])
    outs = outs.reshape(N_CORES, P, NSEG, CH, DV).transpose(0, 2, 3, 1, 4)
    return np.ascontiguousarray(
        outs.reshape(N, DV).astype(np.float32))


def kernel(Q, K, V, omega, num_batch, batch_seg):
    nc = _get_nc()
    in_maps = make_in_maps(Q, K, V, omega)
    res = run_bass_kernel_spmd(nc, in_maps, core_ids=list(range(N_CORES)))
    return unpack_out(res)


# revision 14
# speedup vs baseline: 1.3253x; 1.0372x over previous
"""Trainium2 Bass kernel for segmented linear (performer-style) attention.

Problem: nn_Attention_43550968382196 (sparse_attention).
  N=32768 tokens in 64 contiguous equal segments of 512, d_qk=128, d_v=256,
  m=256 random features.  Per segment:
     phi_q = (exp(Uq - hq - rowmax(Uq)) + eps) / sqrt(m)
     phi_k = (exp(Uk - hk - segmax(Uk)) + eps) / sqrt(m)
     out   = (phi_q @ (phi_k^T V)) / (phi_q . sum(phi_k) + 1e-8)

Device math (equivalent to the reference up to rounding):
  * 1/sqrt(m) cancels in the ratio -> unscaled phi, eps_norm' = 1e-8*m.
  * Both exps run RAW (no bias): exp is monotone, so
    rowmax(exp U) == exp(rowmax U) and the stabilizers become
    multiplicative post-factors.
  * Q side: qp = exp(Uq) * s + eps with s = exp(-hq)/rowmax(exp Uq)
    (one fused tensor_scalar per chunk).
  * K side: exp(-hk) is folded into V on the HOST (V' = exp(-hk)*[V|1|0]),
    so KV = exp(Uk)^T @ V' = (exp(Uk-hk))^T [V|1|0] directly.  The
    segment-max normalizer cancels in the ratio; the phi-eps term needs
    eps*exp(segmax), added as a rank-1 PE-accumulated correction
    KV += (eps*g) * ones (x) Vsum with Vsum precomputed on the host and
    g = allreduce-max(exp Uk) from a GpSimd partition all-reduce.  The
    normalizer eps becomes (1e-8*m)*g, added per row.
  * All big matmuls run in bf16 (host-prerounded inputs; device exps and
    copies produce bf16 operands); PSUM accumulation stays fp32.

DMA strategy: the per-core DMA throughput is limited by descriptor
processing (~280 ns per descriptor, 16 engines; one descriptor per
partition line), NOT by HBM bandwidth.  So every transfer is one big DMA
with maximal per-partition contiguous lines: K^T (+omega +identity packed
as extra columns), Q^T (+exp(-hq)), whole V in device layout (16.5 KB
lines), and the output staged in SBUF and stored in two 4-segment DMAs
(8 KB lines).  ~640 descriptors total vs ~3200 for the naive layout.

Sharding: 64 segments split 8-per-core across 8 NeuronCores (data parallel,
no collectives).  Per core the 8 segments run through a 2-deep software
pipeline: iteration s queues U-matmuls(s), KV+transpose(s-1), output(s-2)
back-to-back on the PE so it never waits on the exp/copy chains.
"""

import math
import os
import sys

for _p in ("/opt/trn_rl_repo",):
    if _p not in sys.path and os.path.isdir(_p):
        sys.path.insert(0, _p)

import numpy as np
import ml_dtypes

import concourse.bass as bass
import concourse.bacc as bacc
import concourse.tile as tile
from concourse import mybir
from concourse import bass_isa
from concourse.bass_utils import run_bass_kernel_spmd

F32 = mybir.dt.float32
BF16 = mybir.dt.bfloat16
AF = mybir.ActivationFunctionType
ALU = mybir.AluOpType
AX = mybir.AxisListType
RED = bass_isa.ReduceOp

N_CORES = 8
N = 32768
D = 128          # qk dim
M = 256          # features
DV = 256         # v dim
DVA = 258        # device V columns: [V | 1 | 0]
P = 128          # partitions / tokens per chunk
NSEG = 8         # segments per core
CH = 4           # chunks per segment
MC = 2           # m chunks (256 / 128)
TOK = NSEG * 512
HSEG = NSEG // 2          # segments per output-store batch

KTX = TOK + M + P         # K^T cols + omega + identity
QTX = TOK + NSEG * CH     # Q^T cols + exp(-hq)

EPS_PHI = 1e-4
EPS_NORM2 = 1e-8 * M
H_SCALE = 1.0 / (2.0 * math.sqrt(D))


def build_nc():
    nc = bacc.Bacc("TRN2", target_bir_lowering=False, debug=False)

    KTd = nc.declare_dram_parameter("KTX", [D, KTX], BF16, isOutput=False)
    QTd = nc.declare_dram_parameter("QTX", [D, QTX], BF16, isOutput=False)
    Vd = nc.declare_dram_parameter("V", [P, NSEG * CH * DVA], BF16,
                                   isOutput=False)
    VSd = nc.declare_dram_parameter("vsum", [1, NSEG * DVA], F32,
                                    isOutput=False)
    Od = nc.declare_dram_parameter("out", [P, NSEG * CH * DV], BF16,
                                   isOutput=True)

    with tile.TileContext(nc) as tc:
        with (
            tc.tile_pool(name="const", bufs=1) as const,
            tc.tile_pool(name="sb", bufs=2) as sb,
            tc.tile_pool(name="sm", bufs=4) as sm,
            tc.tile_pool(name="ps", bufs=1, space="PSUM") as ps,
        ):
            # one descriptor-friendly DMA per tensor (see module docstring)
            kT_x = const.tile([D, KTX], BF16, name="kT_x")
            nc.sync.dma_start(kT_x[:, :], KTd[:, :])
            qT_x = const.tile([D, QTX], BF16, name="qT_x")
            nc.sync.dma_start(qT_x[:, :], QTd[:, :])
            v_all = const.tile([P, NSEG, CH, DVA], BF16, name="v_all")
            nc.sync.dma_start(
                v_all[:, :, :, :],
                Vd[:, :].rearrange("p (s c d) -> p s c d", s=NSEG, c=CH))
            vsum_all = const.tile([1, NSEG, DVA], F32, name="vsum_all")
            nc.sync.dma_start(
                vsum_all[:, :, :],
                VSd[:, :].rearrange("p (s d) -> p s d", s=NSEG))
            onesr_t = const.tile([1, P], BF16, name="onesr_t")
            nc.vector.memset(onesr_t[:, :], 1.0)

            kT_all = kT_x[:, 0:TOK]
            omega_t = kT_x[:, TOK:TOK + M]
            ident_r = kT_x[:, TOK + M:TOK + M + P]
            qT_all = qT_x[:, 0:TOK]
            ehq_all = qT_x[:, TOK:QTX].rearrange("p (s c) -> p s c", s=NSEG)

            # output staging: two 4-segment batches, stored as 8KB lines
            ost = [const.tile([P, HSEG, CH, DV], BF16, name=f"ost{h}")
                   for h in range(2)]

            stA = {}
            stB = {}

            def head(s):
                """U matmuls + exps + maxes + eps factors for segment s."""
                uk = ps.tile([P, CH, M], F32, name=f"uk{s}", tag="uk", bufs=1)
                uq = ps.tile([P, CH, M], F32, name=f"uq{s}", tag="uq", bufs=1)
                for c in range(CH):
                    nc.tensor.matmul(uk[:, c, :],
                                     kT_all[:, bass.ts(s * CH + c, P)],
                                     omega_t[:, :])
                for c in range(CH):
                    nc.tensor.matmul(uq[:, c, :],
                                     qT_all[:, bass.ts(s * CH + c, P)],
                                     omega_t[:, :])

                ek = sb.tile([P, CH, M], BF16, name=f"ek{s}", tag="ek",
                             bufs=3)
                nc.scalar.activation(ek[:, :, :], uk[:, :, :], AF.Exp)
                eq = sb.tile([P, CH, M], BF16, name=f"eq{s}", tag="eq",
                             bufs=3)
                nc.scalar.activation(eq[:, :, :], uq[:, :, :], AF.Exp)

                # K segment max -> g = exp(segmax) on every partition
                kmx = sm.tile([P, 1], F32, name=f"kmx{s}", tag="kmx")
                nc.vector.tensor_reduce(kmx[:, :], ek[:, :, :], axis=AX.XY,
                                        op=ALU.max)
                gmax = sm.tile([P, 1], F32, name=f"gmax{s}", tag="gmax")
                nc.gpsimd.partition_all_reduce(gmax[:, 0:1], kmx[:, 0:1],
                                               channels=P, reduce_op=RED.max)
                cen = sm.tile([P, 1], F32, name=f"cen{s}", tag="cen")
                nc.vector.tensor_scalar_mul(cen[:, :], gmax[:, :], EPS_NORM2)
                cvs = sm.tile([1, DVA], BF16, name=f"cvs{s}", tag="cvs")
                nc.vector.tensor_scalar(cvs[:, :], vsum_all[0:1, s, :],
                                        gmax[0:1, 0:1], EPS_PHI,
                                        op0=ALU.mult, op1=ALU.mult)

                # Q per-row scale s = exp(-hq) / rowmax(exp Uq)
                qmx = sm.tile([P, CH], BF16, name=f"qmx{s}", tag="qmx")
                nc.vector.tensor_reduce(qmx[:, :], eq[:, :, :], axis=AX.X,
                                        op=ALU.max)
                rq = sm.tile([P, CH], F32, name=f"rq{s}", tag="rq")
                nc.vector.reciprocal(rq[:, :], qmx[:, :])
                sq = sm.tile([P, CH], F32, name=f"sq{s}", tag="sq")
                nc.vector.tensor_tensor(sq[:, :], ehq_all[:, s, :], rq[:, :],
                                        op=ALU.mult)
                qp = sb.tile([P, CH, M], BF16, name=f"qp{s}", tag="qp",
                             bufs=3)
                for c in range(CH):
                    nc.vector.tensor_scalar(qp[:, c, :], eq[:, c, :],
                                            sq[:, c:c + 1], EPS_PHI,
                                            op0=ALU.mult, op1=ALU.add)
                stA[s] = (ek, qp, cvs, cen)

            def mid(s):
                """KV matmuls + Qp transposes for segment s."""
                ek, qp, cvs, cen = stA.pop(s)
                kv_sb = sb.tile([P, MC, DVA], BF16, name=f"kvsb{s}",
                                tag="kvsb", bufs=3)
                qpT_sb = sb.tile([P, MC, 512], BF16, name=f"qpTsb{s}",
                                 tag="qpTsb", bufs=3)
                for mc in range(MC):
                    kvp = ps.tile([P, 512], F32, name=f"kv{s}_{mc}", tag="W",
                                  bufs=4)
                    for c in range(CH):
                        nc.tensor.matmul(kvp[:, 0:DVA],
                                         ek[:, c, bass.ts(mc, P)],
                                         v_all[:, s, c, :],
                                         start=(c == 0), stop=False)
                    nc.tensor.matmul(kvp[:, 0:DVA], onesr_t[0:1, :],
                                     cvs[0:1, :], start=False, stop=True)
                    nc.vector.tensor_copy(kv_sb[:, mc, :], kvp[:, 0:DVA])
                    qpTp = ps.tile([P, 512], BF16, name=f"qpT{s}_{mc}",
                                   tag="W", bufs=4)
                    for c in range(CH):
                        nc.tensor.transpose(qpTp[:, bass.ts(c, P)],
                                            qp[:, c, bass.ts(mc, P)],
                                            ident_r[:, :])
                    if mc == 0:
                        nc.scalar.copy(qpT_sb[:, mc, :], qpTp[:, :])
                    else:
                        nc.vector.tensor_copy(qpT_sb[:, mc, :], qpTp[:, :])
                stB[s] = (kv_sb, qpT_sb, cen)

            def tail(s):
                """Numerator, normalization and staged output for segment s."""
                kv_sb, qpT_sb, cen = stB.pop(s)
                ot = ost[s // HSEG]
                for c in range(CH):
                    nm = ps.tile([P, 512], F32, name=f"nm{s}_{c}",
                                 tag="W", bufs=4)
                    for mc in range(MC):
                        nc.tensor.matmul(nm[:, 0:DVA],
                                         qpT_sb[:, mc, bass.ts(c, P)],
                                         kv_sb[:, mc, :],
                                         start=(mc == 0),
                                         stop=(mc == MC - 1))
                    den = sm.tile([P, 1], F32, name=f"den{s}_{c}", tag="den")
                    nc.vector.tensor_tensor(den[:, :], nm[:, DV:DV + 1],
                                            cen[:, 0:1], op=ALU.add)
                    rr = sm.tile([P, 1], F32, name=f"rr{s}_{c}", tag="rr")
                    nc.vector.reciprocal(rr[:, :], den[:, :])
                    nc.scalar.activation(ot[:, s % HSEG, c, :], nm[:, 0:DV],
                                         AF.Copy, scale=rr[:, 0:1])
                if s % HSEG == HSEG - 1:
                    h = s // HSEG
                    nc.sync.dma_start(
                        Od[:, bass.ts(h, HSEG * CH * DV)]
                        .rearrange("p (s c d) -> p s c d", s=HSEG, c=CH),
                        ot[:, :, :, :])

            # 2-deep software pipeline (see module docstring)
            for s in range(NSEG):
                head(s)
                if s >= 1:
                    mid(s - 1)
                if s >= 2:
                    tail(s - 2)
            mid(NSEG - 1)
            tail(NSEG - 2)
            tail(NSEG - 1)

    nc.compile()
    return nc


_NC_CACHE = {}


def _get_nc():
    if "nc" not in _NC_CACHE:
        _NC_CACHE["nc"] = build_nc()
    return _NC_CACHE["nc"]


def _bf16(x):
    return np.ascontiguousarray(x.astype(ml_dtypes.bfloat16))


def make_in_maps(Q, K, V, omega):
    Q = np.ascontiguousarray(np.asarray(Q, dtype=np.float32))
    K = np.ascontiguousarray(np.asarray(K, dtype=np.float32))
    hscale = np.float32(H_SCALE)
    ehq = np.exp(-(Q * Q).sum(axis=1) * hscale)          # exp(-hq)  [N]
    ehk = np.exp(-(K * K).sum(axis=1) * hscale)          # exp(-hk)  [N]
    V = np.asarray(V, dtype=np.float32)
    Vaug = np.zeros((V.shape[0], DVA), np.float32)
    Vaug[:, :DV] = V
    Vaug[:, DV] = 1.0
    # per-segment column sums of [V | 1 | 0] (fp32, host-side)
    vsum = Vaug.reshape(N // 512, 512, DVA).sum(axis=1)  # [64, DVA]
    Vp16 = _bf16(Vaug * ehk[:, None])                    # exp(-hk)-folded V
    # device layout: [P, (s c d)] per core
    Vdev = Vp16.reshape(N_CORES, NSEG, CH, P, DVA).transpose(0, 3, 1, 2, 4)
    Vdev = np.ascontiguousarray(
        Vdev.reshape(N_CORES, P, NSEG * CH * DVA))
    omega = np.asarray(omega, dtype=np.float32)
    omega16 = (omega * np.float32(D ** -0.25)).astype(ml_dtypes.bfloat16)
    ident16 = np.eye(P, dtype=ml_dtypes.bfloat16)
    QT16 = _bf16(Q.T)
    KT16 = _bf16(K.T)
    ehq16 = ehq.astype(ml_dtypes.bfloat16)
    in_maps = []
    for c in range(N_CORES):
        sl = slice(c * TOK, (c + 1) * TOK)
        ktx = np.concatenate([KT16[:, sl], omega16, ident16], axis=1)
        ehq_dev = (ehq16[sl].reshape(NSEG, CH, P)
                   .transpose(2, 0, 1).reshape(P, NSEG * CH))
        qtx = np.concatenate([QT16[:, sl], ehq_dev], axis=1)
        in_maps.append({
            "KTX": np.ascontiguousarray(ktx),
            "QTX": np.ascontiguousarray(qtx),
            "V": Vdev[c],
            "vsum": np.ascontiguousarray(
                vsum[c * NSEG:(c + 1) * NSEG].reshape(1, NSEG * DVA)),
        })
    return in_maps


def unpack_out(res):
    # out arrives in device layout [P, (s c d)] bf16 per core
    outs = np.stack([np.asarray(res.results[c]["out"])
                     for c in range(N_CORES)])
    outs = outs.reshape(N_CORES, P, NSEG, CH, DV).transpose(0, 2, 3, 1, 4)
    return np.ascontiguousarray(
        outs.reshape(N, DV).astype(np.float32))


def kernel(Q, K, V, omega, num_batch, batch_seg):
    nc = _get_nc()
    in_maps = make_in_maps(Q, K, V, omega)
    res = run_bass_kernel_spmd(nc, in_maps, core_ids=list(range(N_CORES)))
    return unpack_out(res)
